# revision 37
# baseline (speedup 1.0000x reference)
"""Expert-parallel MoE MLP kernel for Trainium2 (8 NeuronCores).

Problem: out[b,e,n,d] = gelu(x[b,e] @ w1[e] + b1[e]) @ w2[e] + b2[e]
Shapes: x [2,8,1024,1024] f32, w1 [8,1024,4096], b1 [8,4096],
        w2 [8,4096,1024], b2 [8,1024].

Sharding: expert e -> core e. Each core runs a 2048-token MLP:
  [2048,1024] @ [1024,4096] -> gelu -> @ [4096,1024] -> [2048,1024]

Device-side layout: activations live transposed ([feature, token]) so the
contraction dim is always the SBUF partition dim:
  phase 1: psum[h_tile, t] += w1[d_tile, h_tile].T @ xT[d_tile, t]
  phase 2: psum[d_tile, t] += w2[h_tile, d_tile].T @ hT[h_tile, t]
Host transposes x on the way in and out on the way back (part of
shard/unshard), so the device does zero transposes.

All matmul inputs are bf16 (fp32 PSUM accumulation); GELU (tanh approx,
matching jax.nn.gelu default) fused with the b1 add on ScalarE.

Perf notes (from NTFF traces):
- steady state is a perfect 216 ns/matmul stream; all overhead is at the
  two ends of the kernel.
- w1/xt/b1/b2 are pre-packed on the host so every DMA descriptor reads
  >=2KB contiguous per partition (naive strided layouts produced 390-byte
  w1 packets and 5k+ 4-byte bias packets, clogging the weight queue and
  delaying the first matmul to ~21us).
- x block 0 is split across the scalar + sync DMA queues (w1 chunk 0
  triggers first on sync; biases ride gpsimd) so the lead-in-critical
  1.25MB lands at the ~360 GB/s aggregate floor; later x blocks are
  dependency-anchored to the previous block's phase-1 so their 8KB
  packets don't starve the w1 stream.
- `wu` dummy matmuls on zeroed scratch tiles run while input DMAs are in
  flight, so the PE's HAM clock-gate (1.2 GHz cold -> 2.4 GHz after
  ~3.4us busy) is already warm when real data lands.
- the very last d-tile's 32-matmul group is column-split in two, so the
  final 256KB store overlaps the last half-group instead of trailing it.
"""

import sys

for _p in ("/opt/trn_rl_repo",):
    if _p not in sys.path:
        sys.path.insert(0, _p)

import numpy as np
import ml_dtypes

from contextlib import ExitStack

import concourse.bass as bass
import concourse.tile as tile
from concourse import bacc, mybir
from concourse.bass import _add_dep_helper
from concourse.bass_utils import run_bass_kernel_spmd

BF16 = mybir.dt.bfloat16
F32 = mybir.dt.float32

# Full-problem constants (hardcoded per harness contract).
B, E, N, D, H = 2, 8, 1024, 1024, 4096
T = B * N          # tokens per expert/core
TBLK = 512         # tokens per block (= one PSUM bank of fp32)
P = 128


def pack_xt(xT, tblk=TBLK):
    """[d, t] -> [t//tblk, 128, d//128, tblk]; per-partition rows contiguous."""
    d, t = xT.shape
    return np.ascontiguousarray(
        xT.reshape(d // P, P, t // tblk, tblk).transpose(2, 1, 0, 3))


def pack_w1(w1):
    """[d, h] -> [h//128, 128, d//128, 128]; per-partition rows contiguous."""
    d, h = w1.shape
    return np.ascontiguousarray(
        w1.reshape(d // P, P, h // P, P).transpose(2, 1, 0, 3))


def pack_b(b):
    """[f] -> [128, f//128]; partition-major so the DMA is contiguous."""
    return np.ascontiguousarray(b.reshape(-1, P).T)


def build_nc(t=T, d=D, h=H, tblk=TBLK, act=None, wu=12, split_x0=True,
             split_last=True):
    """Build the per-core Bass program. All cores run this same program on
    different data (SPMD). wu = PE warm-up matmul count (HAM un-throttle)."""
    if act is None:
        act = mybir.ActivationFunctionType.Gelu_apprx_tanh
    kd = d // P        # contraction tiles for phase 1
    nh = h // P        # h tiles (phase-1 outputs / phase-2 contraction)
    nd = d // P        # d tiles (phase-2 outputs)
    nblk = t // tblk
    # x block-0 split points: scalar gets [0,k2), sync [k2,kd) behind
    # w1c0. (An extra small sync-first xa0 piece for "early partial work"
    # measured 0.3us WORSE — it delays w1c0, which the first matmul also
    # needs.) At small kd (CoreSim shapes) degrade to fewer pieces.
    k1 = 0
    k2 = max(1, (5 * kd) // 8)

    nc = bacc.Bacc("TRN2", target_bir_lowering=False)

    xt_hbm = nc.dram_tensor("xt", [nblk, P, kd, tblk], BF16,
                            kind="ExternalInput").ap()
    w1_hbm = nc.dram_tensor("w1", [nh, P, kd, P], BF16,
                            kind="ExternalInput").ap()
    w2_hbm = nc.dram_tensor("w2", [h, d], BF16, kind="ExternalInput").ap()
    b1_hbm = nc.dram_tensor("b1", [P, nh], F32, kind="ExternalInput").ap()
    b2_hbm = nc.dram_tensor("b2", [P, nd], F32, kind="ExternalInput").ap()
    # Output is stored bf16 (host upcasts to f32): the final bias-add runs
    # at DVE 2x 16-bit rate and the last store is half the bytes, pulling
    # the teardown gate ~0.5us earlier. Adds ~0.23% RMS quantization
    # (total rel err ~4e-3 vs the 2e-2 gate).
    out_hbm = nc.dram_tensor("outT", [d, t], BF16, kind="ExternalOutput").ap()

    w2_v = w2_hbm.rearrange("(kh p) d -> p kh d", p=P)

    with tile.TileContext(nc) as tc, ExitStack() as ctx:
        w1_pool = ctx.enter_context(tc.tile_pool(name="w1", bufs=nh))
        w2_pool = ctx.enter_context(tc.tile_pool(name="w2", bufs=nh))
        x_pool = ctx.enter_context(tc.tile_pool(name="x", bufs=2))
        h_pool = ctx.enter_context(tc.tile_pool(name="h", bufs=nh + 2))
        o_pool = ctx.enter_context(tc.tile_pool(name="o", bufs=4))
        c_pool = ctx.enter_context(tc.tile_pool(name="c", bufs=1))
        ps1 = ctx.enter_context(tc.tile_pool(name="ps1", bufs=2, space="PSUM"))
        ps2 = ctx.enter_context(tc.tile_pool(name="ps2", bufs=2, space="PSUM"))
        psw = ctx.enter_context(tc.tile_pool(name="psw", bufs=1, space="PSUM"))

        # PE warm-up: matmuls on zeroed scratch tiles, issued before any
        # data-dependent matmul so they run while the input DMAs stream.
        # All accumulate into one never-stored PSUM bank (accumulation
        # keeps every matmul live; zeros keep it numerically inert).
        if wu:
            wsrc = c_pool.tile([P, tblk], BF16)
            nc.vector.memset(wsrc, 0)
            wwt = c_pool.tile([P, P], BF16)
            nc.vector.memset(wwt, 0)
            psw_t = psw.tile([P, tblk], F32)
            for i in range(wu):
                nc.tensor.matmul(psw_t, wwt, wsrc,
                                 start=(i == 0), stop=(i == wu - 1))

        # Lead-in critical path. Queue facts (from NTFF traces): each
        # dma_start trigger costs ~0.6us on its issuing engine; the sync
        # and scalar queues (HW descriptor gen) see first data ~2-3.5us
        # after the trigger, gpsimd (SW descriptor gen) ~5.5us; queues
        # fair-share HBM per packet, so 8KB-packet streams get ~4x the
        # bytes of 2KB-packet streams.
        #
        # The first matmul needs w1 chunk 0 + x block 0, so: w1c0 is the
        # first trigger on sync, x block 0 is split between scalar
        # (chunks 0..kh-1) and sync (kh..), and everything else stays out
        # of the way (biases on gpsimd; x blocks 1+ are anchored to the
        # previous block's phase-1 progress so their 8KB packets don't
        # starve the w1 stream during the lead-in).
        x0_pieces = []            # (lo, hi, tile) covering block 0's kd dim

        def x0_piece(nm, eng, lo, hi):
            if hi <= lo:
                return
            xp = x_pool.tile([P, hi - lo, tblk], BF16, name=nm, tag=nm,
                             bufs=1)
            eng.dma_start(out=xp, in_=xt_hbm[0, :, lo:hi, :])
            x0_pieces.append((lo, hi, xp))

        if split_x0:
            x0_piece("xa0", nc.sync, 0, k1)

        w1_t = []
        wt = w1_pool.tile([P, kd, P], BF16, name="w1c0", tag="w1c")
        nc.sync.dma_start(out=wt, in_=w1_hbm[0])
        w1_t.append(wt)

        if split_x0:
            x0_piece("xa1", nc.scalar, k1, k2)
            x0_piece("xb", nc.sync, k2, kd)

        for ih in range(1, nh):
            wt = w1_pool.tile([P, kd, P], BF16, name=f"w1c{ih}", tag="w1c")
            nc.sync.dma_start(out=wt, in_=w1_hbm[ih])
            w1_t.append(wt)

        # Biases, resident (host pre-transposed -> contiguous DMA). On
        # gpsimd after xc; needed only by the first ACTIVATE (~1.7us after
        # the first matmul group), so off the critical path there.
        b1_sb = c_pool.tile([P, nh], F32)
        nc.gpsimd.dma_start(out=b1_sb, in_=b1_hbm)
        b2_sb = c_pool.tile([P, nd], F32)
        nc.gpsimd.dma_start(out=b2_sb, in_=b2_hbm)

        w2_t = []
        for ikh in range(nh):
            wt = w2_pool.tile([P, d], BF16)
            nc.sync.dma_start(out=wt, in_=w2_v[:, ikh, :])
            w2_t.append(wt)

        gelu = act
        anchor = None
        for ib in range(nblk):
            if ib == 0 and split_x0:
                def xsl(ik):
                    for lo, hi, xp in x0_pieces:
                        if lo <= ik < hi:
                            return xp[:, ik - lo, :]
                    raise AssertionError(ik)
            else:
                xt = x_pool.tile([P, kd, tblk], BF16)
                dbi = nc.gpsimd.dma_start(out=xt, in_=xt_hbm[ib])
                if anchor is not None:
                    _add_dep_helper(dbi.ins, anchor.ins, sync=True,
                                    reason="stagger x prefetch off lead-in")

                def xsl(ik, _xt=xt):
                    return _xt[:, ik, :]

            # phase 1: hT[h_tile] = gelu(w1.T @ xT + b1)
            ht = []
            for ih in range(nh):
                ps = ps1.tile([P, tblk], F32)
                for ik in range(kd):
                    nc.tensor.matmul(
                        ps, w1_t[ih][:, ik, :], xsl(ik),
                        start=(ik == 0), stop=(ik == kd - 1),
                    )
                hs = h_pool.tile([P, tblk], BF16)
                abi = nc.scalar.activation(hs, ps, gelu,
                                           bias=b1_sb[:, ih:ih + 1])
                if ih == 8:
                    anchor = abi
                ht.append(hs)

            # phase 2: outT[d_tile] = w2.T @ hT + b2
            for idt in range(nd):
                last = (ib == nblk - 1 and idt == nd - 1 and split_last)
                if not last:
                    ps = ps2.tile([P, tblk], F32)
                    for ikh in range(nh):
                        nc.tensor.matmul(
                            ps, w2_t[ikh][:, idt * P:(idt + 1) * P], ht[ikh],
                            start=(ikh == 0), stop=(ikh == nh - 1),
                        )
                    ob = o_pool.tile([P, tblk], BF16)
                    nc.vector.tensor_scalar_add(ob, ps, b2_sb[:, idt:idt + 1])
                    nc.scalar.dma_start(
                        out=out_hbm[idt * P:(idt + 1) * P,
                                    ib * tblk:(ib + 1) * tblk],
                        in_=ob,
                    )
                else:
                    # Final tile: column-split the matmul group so the
                    # first half's store overlaps the second half-group.
                    hb = tblk // 2
                    for c in range(2):
                        csl = slice(c * hb, (c + 1) * hb)
                        ps = ps2.tile([P, hb], F32, name=f"psl{c}", bufs=1)
                        for ikh in range(nh):
                            nc.tensor.matmul(
                                ps, w2_t[ikh][:, idt * P:(idt + 1) * P],
                                ht[ikh][:, csl],
                                start=(ikh == 0), stop=(ikh == nh - 1),
                            )
                        ob = o_pool.tile([P, hb], BF16, name=f"obl{c}",
                                         tag="obl")
                        nc.vector.tensor_scalar_add(ob, ps,
                                                    b2_sb[:, idt:idt + 1])
                        nc.scalar.dma_start(
                            out=out_hbm[idt * P:(idt + 1) * P,
                                        ib * tblk + c * hb:
                                        ib * tblk + (c + 1) * hb],
                            in_=ob,
                        )

    nc.compile()
    return nc


_NC_CACHE = {}


def _get_nc():
    if "nc" not in _NC_CACHE:
        _NC_CACHE["nc"] = build_nc()
    return _NC_CACHE["nc"]


def make_in_maps(x, w1, b1, w2, b2):
    bf16 = ml_dtypes.bfloat16
    in_maps = []
    for e in range(E):
        xe = np.asarray(x[:, e], dtype=np.float32).reshape(T, D)
        in_maps.append({
            "xt": pack_xt(np.ascontiguousarray(xe.T).astype(bf16)),
            "w1": pack_w1(np.asarray(w1[e], dtype=np.float32).astype(bf16)),
            "w2": np.asarray(w2[e], dtype=np.float32).astype(bf16),
            "b1": pack_b(np.asarray(b1[e], np.float32)),
            "b2": pack_b(np.asarray(b2[e], np.float32)),
        })
    return in_maps


def kernel(x, w1, b1, w2, b2):
    nc = _get_nc()
    in_maps = make_in_maps(x, w1, b1, w2, b2)

    res = run_bass_kernel_spmd(nc, in_maps, core_ids=list(range(E)))

    out = np.empty((B, E, N, D), dtype=np.float32)
    for e in range(E):
        ot = np.asarray(res.results[e]["outT"]).astype(np.float32)  # [D, T]
        out[:, e] = ot.T.reshape(B, N, D)
    return out
